# revision 5
# baseline (speedup 1.0000x reference)
"""Trainium2 Bass kernel: CausalCrossAttention (GroupNorm + Q proj + block-causal
cross-attention over a small context + out proj + residual).

Sharding: 8 cores, each owns one (batch b, frame-residue r) pair:
  b = core // 4, r = core % 4, frames t = r + 4*f for f in 0..3.
GroupNorm normalizes each (b, t) frame independently over (16ch x H*W), and
attention key/value come from the (tiny) per-batch context, so every core's
work is fully local -- no collectives.  The block-causal mask is shipped as a
per-core additive bias tensor so all cores run the identical SPMD graph.

All heavy matmuls run in bf16 on the TensorEngine (f32 accumulation in PSUM);
GroupNorm statistics are computed in f32 (bn_stats/bn_aggr + tiny f32 matmuls
to fold 16-channel groups across partitions).  rsqrt(var+eps) is computed as
exp(-0.5*ln(var+eps)) so the ScalarEngine needs only one activation table set
(ln/exp, with copy/identity as fillers) for the entire kernel.
"""

import numpy as np

import concourse.bass as bass
import concourse.bacc as bacc
import concourse.mybir as mybir
import concourse.tile as tile
from concourse.bass_utils import run_bass_kernel_spmd
from concourse.masks import make_identity

# Problem shape (fixed by the harness).
B, C, T, H, W = 2, 512, 16, 32, 32
HW = H * W            # 1024 query positions per frame
S, D = 64, 1024       # context length, context dim
G = 32                # groupnorm groups
CPG = C // G          # 16 channels per group
NCORES = 8
FPC = (B * T) // NCORES   # 4 frames per core
NCH = C // 128        # 4 channel chunks of 128
NDCH = D // 128       # 8 context-dim chunks
EPS = 1e-5
SCALE = float(C) ** -0.5
NEGINF = -1e9

F32 = mybir.dt.float32
BF16 = mybir.dt.bfloat16

LAST_RESULT = None        # BassKernelResults of the most recent run (for test.py)
_GRAPH_CACHE = {}


def _chunked(dram_ap):
    """[N*128, ...] dram AP -> [128, N, ...] with channel = n*128 + p."""
    return dram_ap.rearrange("(a p) w -> p a w", p=128)


def _build(with_bkv: bool, with_bo: bool) -> bass.Bass:
    nc = bacc.Bacc()

    x_d = nc.declare_dram_parameter("x", [C, FPC, HW], F32, isOutput=False)
    ctxT_d = nc.declare_dram_parameter("ctxT", [D, S], F32, isOutput=False)
    wqT_d = nc.declare_dram_parameter("wqT", [C, C], F32, isOutput=False)
    wkvT_d = nc.declare_dram_parameter("wkvT", [D, 2 * C], F32, isOutput=False)
    woT_d = nc.declare_dram_parameter("woT", [C, C], F32, isOutput=False)
    gammaT_d = nc.declare_dram_parameter("gammaT", [128, NCH], F32, isOutput=False)
    betaT_d = nc.declare_dram_parameter("betaT", [128, NCH], F32, isOutput=False)
    bqT_d = nc.declare_dram_parameter("bqT", [128, NCH], F32, isOutput=False)
    bkv_d = nc.declare_dram_parameter("bkv", [1, 2 * C], F32, isOutput=False)
    bo_d = nc.declare_dram_parameter("bo", [1, C], F32, isOutput=False)
    mask_d = nc.declare_dram_parameter("mask", [FPC, 1, S], F32, isOutput=False)
    gmat_d = nc.declare_dram_parameter("gmat", [128, 8], F32, isOutput=False)
    emat_d = nc.declare_dram_parameter("emat", [8, 128], F32, isOutput=False)
    out_d = nc.declare_dram_parameter("out", [C, FPC, HW], F32, isOutput=True)

    Identity = mybir.ActivationFunctionType.Identity
    Copy = mybir.ActivationFunctionType.Copy
    Exp = mybir.ActivationFunctionType.Exp
    Ln = mybir.ActivationFunctionType.Ln

    with tile.TileContext(nc) as tc:
        with (
            tc.tile_pool(name="weights", bufs=1) as wp,
            tc.tile_pool(name="stage", bufs=2) as stage,
            tc.tile_pool(name="xp", bufs=2) as xp,
            tc.tile_pool(name="hp", bufs=2) as hp,
            tc.tile_pool(name="qp", bufs=2) as qp,
            tc.tile_pool(name="op", bufs=2) as op_,
            tc.tile_pool(name="rp", bufs=2) as rp,
            tc.tile_pool(name="small", bufs=3) as small,
            tc.tile_pool(name="psA", bufs=3, space="PSUM") as psA,
            tc.tile_pool(name="psB", bufs=2, space="PSUM") as psB,
        ):
            # ---------------- constants & weights ----------------
            gammaT_sb = wp.tile([128, NCH], F32)
            betaT_sb = wp.tile([128, NCH], F32)
            bqT_sb = wp.tile([128, NCH], F32)
            gmat_sb = wp.tile([128, 8], F32)
            emat_sb = wp.tile([8, 128], F32)
            mask_sb = wp.tile([128, FPC, 1, S], F32)
            identity = wp.tile([128, 128], BF16)
            eps_sb = wp.tile([128, 1], F32)

            nc.gpsimd.dma_start(out=gammaT_sb[:], in_=gammaT_d[:, :])
            nc.gpsimd.dma_start(out=betaT_sb[:], in_=betaT_d[:, :])
            nc.gpsimd.dma_start(out=bqT_sb[:], in_=bqT_d[:, :])
            nc.gpsimd.dma_start(out=gmat_sb[:], in_=gmat_d[:, :])
            nc.gpsimd.dma_start(out=emat_sb[:], in_=emat_d[:, :])
            m_ap = mask_d[:, :, :]
            nc.gpsimd.dma_start(
                out=mask_sb[:],
                in_=bass.AP(tensor=m_ap.tensor, offset=m_ap.offset,
                            ap=[[0, 128]] + list(m_ap.ap)),
            )
            make_identity(nc, identity[:])
            nc.vector.memset(eps_sb[:], EPS)

            wq_bf = wp.tile([128, NCH, C], BF16)
            wkv_bf = wp.tile([128, NDCH, 2 * C], BF16)
            wo_bf = wp.tile([128, NCH, C], BF16)
            ctx_bf = wp.tile([128, NDCH, S], BF16)

            st = stage.tile([128, NCH, C], F32, tag="stage")
            nc.gpsimd.dma_start(out=st[:], in_=_chunked(wqT_d[:, :]))
            nc.gpsimd.tensor_copy(out=wq_bf[:], in_=st[:])

            stc = stage.tile([128, NDCH, S], F32, tag="stage")
            nc.gpsimd.dma_start(out=stc[:], in_=_chunked(ctxT_d[:, :]))
            nc.gpsimd.tensor_copy(out=ctx_bf[:], in_=stc[:])

            wkvT_c = _chunked(wkvT_d[:, :])  # [128, 8, 1024]
            for hh in range(2):
                stw = stage.tile([128, NDCH // 2, 2 * C], F32, tag="stage")
                nc.gpsimd.dma_start(out=stw[:], in_=wkvT_c[:, hh * 4:(hh + 1) * 4, :])
                nc.gpsimd.tensor_copy(
                    out=wkv_bf[:, hh * 4:(hh + 1) * 4, :], in_=stw[:])

            sto = stage.tile([128, NCH, C], F32, tag="stage")
            nc.gpsimd.dma_start(out=sto[:], in_=_chunked(woT_d[:, :]))
            nc.gpsimd.tensor_copy(out=wo_bf[:], in_=sto[:])

            if with_bkv:
                ones64 = wp.tile([1, S], BF16)
                nc.vector.memset(ones64[:], 1.0)
                stb = small.tile([1, 2 * C], F32)
                nc.gpsimd.dma_start(out=stb[:], in_=bkv_d[:, :])
                bkv_bf = wp.tile([1, 2 * C], BF16)
                nc.gpsimd.tensor_copy(out=bkv_bf[:], in_=stb[:])
            if with_bo:
                ones512 = wp.tile([1, 512], BF16)
                nc.vector.memset(ones512[:], 1.0)
                sbo = small.tile([1, C], F32)
                nc.gpsimd.dma_start(out=sbo[:], in_=bo_d[:, :])
                bo_bf = wp.tile([1, C], BF16)
                nc.gpsimd.tensor_copy(out=bo_bf[:], in_=sbo[:])

            # ---------------- kv projection (once per core) ----------------
            # kv[s, o] = sum_d ctx[s, d] * wkv[o, d] (+ bkv);  k = kv[:, :C], v = kv[:, C:]
            kT_sb = wp.tile([128, NCH, S], BF16)   # k transposed: [c, s]
            v_sb = wp.tile([64, C], BF16)          # v: [s, c]
            for half in range(2):
                psum_kv = psB.tile([64, 512], F32, tag="ps_small")
                for dci in range(NDCH):
                    nc.tensor.matmul(
                        psum_kv[:],
                        lhsT=ctx_bf[:, dci, :],
                        rhs=wkv_bf[:, dci, half * 512:(half + 1) * 512],
                        start=(dci == 0),
                        stop=(dci == NDCH - 1 and not with_bkv),
                    )
                if with_bkv:
                    nc.tensor.matmul(
                        psum_kv[:], lhsT=ones64[:],
                        rhs=bkv_bf[:, half * 512:(half + 1) * 512],
                        start=False, stop=True,
                    )
                if half == 0:
                    k_sb = small.tile([64, 512], BF16)
                    nc.scalar.activation(out=k_sb[:], in_=psum_kv[:], func=Copy)
                    psum_kT = psB.tile([128, NCH, S], BF16, tag="ps_small")
                    for ci in range(NCH):
                        nc.tensor.transpose(
                            psum_kT[:, ci, :],
                            k_sb[:, ci * 128:(ci + 1) * 128],
                            identity[:64, :64],
                        )
                    nc.scalar.activation(out=kT_sb[:], in_=psum_kT[:], func=Copy)
                else:
                    nc.scalar.activation(out=v_sb[:], in_=psum_kv[:], func=Copy)

            # ---------------- per-frame pipeline ----------------
            for f in range(FPC):
                # load x frame: [128, chunk, hw]
                x_sb = xp.tile([128, NCH, HW], F32)
                nc.gpsimd.dma_start(out=x_sb[:], in_=_chunked(x_d[:, f, :]))

                # --- groupnorm statistics ---
                st6 = small.tile([128, NCH, 2, 6], F32)
                mv = small.tile([128, NCH, 2], F32)
                for ci in range(NCH):
                    xv = x_sb[:, ci, :].rearrange("p (a b) -> p a b", a=2)
                    for k2 in range(2):
                        nc.vector.bn_stats(out=st6[:, ci, k2, :], in_=xv[:, k2, :])
                    nc.vector.bn_aggr(out=mv[:, ci, :], in_=st6[:, ci, :, :])
                # E[x^2] per channel: var + mean^2 (in place in mv[:, :, 1])
                msq = small.tile([128, NCH], F32)
                nc.vector.tensor_mul(msq[:], mv[:, :, 0], mv[:, :, 0])
                nc.vector.tensor_add(mv[:, :, 1], mv[:, :, 1], msq[:])
                # fold 16-channel groups across partitions: [128, 8] -> [8, 8]
                psum_g = psB.tile([8, 8], F32, tag="ps_small")
                nc.tensor.matmul(
                    psum_g[:], lhsT=gmat_sb[:],
                    rhs=mv[:].rearrange("p a b -> p (a b)"),
                    start=True, stop=True,
                )
                gs = small.tile([8, NCH, 2], F32)
                nc.scalar.activation(
                    out=gs[:], in_=psum_g[:].rearrange("p (a b) -> p a b", a=NCH),
                    func=Copy)
                gsq = small.tile([8, NCH], F32)
                nc.vector.tensor_mul(gsq[:], gs[:, :, 0], gs[:, :, 0])
                nc.vector.tensor_sub(gs[:, :, 1], gs[:, :, 1], gsq[:])
                # rs = (var+eps)^-1/2 = exp(-0.5*ln(var+eps)) (stays in ln/exp table set)
                nc.scalar.activation(out=gs[:, :, 1], in_=gs[:, :, 1], func=Ln,
                                     bias=eps_sb[:8, :], scale=1.0)
                nc.scalar.activation(out=gs[:, :, 1], in_=gs[:, :, 1], func=Exp,
                                     scale=-0.5)
                # expand group stats back to channels: [8, 8] -> [128, 8]
                psum_e = psB.tile([128, NCH, 2], F32, tag="ps_small")
                nc.tensor.matmul(
                    psum_e[:].rearrange("p a b -> p (a b)"),
                    lhsT=emat_sb[:], rhs=gs[:].rearrange("p a b -> p (a b)"),
                    start=True, stop=True,
                )
                # per-channel affine: h = a*x + b, a = rs*gamma, b = beta - mu*a
                a_sb = small.tile([128, NCH], F32)
                t_sb = small.tile([128, NCH], F32)
                b_sb = small.tile([128, NCH], F32)
                nc.vector.tensor_mul(a_sb[:], psum_e[:, :, 1], gammaT_sb[:])
                nc.vector.tensor_mul(t_sb[:], psum_e[:, :, 0], a_sb[:])
                nc.vector.tensor_sub(b_sb[:], betaT_sb[:], t_sb[:])

                h_sb = hp.tile([128, NCH, HW], BF16)
                for ci in range(NCH):
                    nc.scalar.activation(
                        out=h_sb[:, ci, :], in_=x_sb[:, ci, :], func=Identity,
                        bias=b_sb[:, ci:ci + 1], scale=a_sb[:, ci:ci + 1])

                # --- q projection: q[oc, p] = sum_c wq[oc, c] h[c, p] + bq ---
                q_sb = qp.tile([128, NCH, HW], BF16)
                for oc in range(NCH):
                    psum_q = psA.tile([128, 2, 512], F32, tag="ps_big")
                    for half in range(2):
                        for ci in range(NCH):
                            nc.tensor.matmul(
                                psum_q[:, half, :],
                                lhsT=wq_bf[:, ci, oc * 128:(oc + 1) * 128],
                                rhs=h_sb[:, ci, half * 512:(half + 1) * 512],
                                start=(ci == 0), stop=(ci == NCH - 1),
                            )
                    nc.scalar.activation(
                        out=q_sb[:, oc, :],
                        in_=psum_q[:].rearrange("p a b -> p (a b)"),
                        func=Identity, bias=bqT_sb[:, oc:oc + 1], scale=1.0)

                # --- scores: [q(8x128), s] with block-causal additive mask ---
                psum_s = psB.tile([128, 8, S], F32, tag="ps_small")
                for j in range(8):
                    for ci in range(NCH):
                        nc.tensor.matmul(
                            psum_s[:, j, :],
                            lhsT=q_sb[:, ci, j * 128:(j + 1) * 128],
                            rhs=kT_sb[:, ci, :],
                            start=(ci == 0), stop=(ci == NCH - 1),
                        )
                sc_sb = small.tile([128, 8, S], F32)
                nc.vector.tensor_add(
                    sc_sb[:], psum_s[:],
                    mask_sb[:, f, :, :].to_broadcast((128, 8, S)))
                # softmax over s (no max-subtraction needed: |scale*scores| is small)
                p_sb = small.tile([128, 8, S], F32)
                nc.scalar.activation(out=p_sb[:], in_=sc_sb[:], func=Exp, scale=SCALE)
                l8 = small.tile([128, 8, 1], F32)
                nc.vector.reduce_sum(l8[:], p_sb[:], axis=mybir.AxisListType.X)
                linv = small.tile([128, 8, 1], F32)
                nc.vector.reciprocal(linv[:], l8[:])
                p_bf = small.tile([128, 8, S], BF16)
                nc.vector.tensor_mul(p_bf[:], p_sb[:], linv[:].to_broadcast((128, 8, S)))

                # --- transpose weights to [s, q] for the PV matmul ---
                psum_wT = psB.tile([64, 8, 128], BF16, tag="ps_small")
                for j in range(8):
                    nc.tensor.transpose(psum_wT[:, j, :], p_bf[:, j, :], identity[:])
                wT_sb = small.tile([64, 8, 128], BF16)
                nc.scalar.activation(out=wT_sb[:], in_=psum_wT[:], func=Copy)
                wT_flat = wT_sb[:].rearrange("p a b -> p (a b)")  # [64, 1024]

                # --- attention output (transposed): o[c, q] = sum_s v[s, c] w[q, s] ---
                oatt = op_.tile([128, NCH, HW], BF16)
                for ci in range(NCH):
                    psum_pv = psA.tile([128, 2, 512], F32, tag="ps_big")
                    for half in range(2):
                        nc.tensor.matmul(
                            psum_pv[:, half, :],
                            lhsT=v_sb[:, ci * 128:(ci + 1) * 128],
                            rhs=wT_flat[:, half * 512:(half + 1) * 512],
                            start=True, stop=True,
                        )
                    nc.scalar.activation(
                        out=oatt[:, ci, :],
                        in_=psum_pv[:].rearrange("p a b -> p (a b)"), func=Copy)

                # --- output projection + residual ---
                out_sb = rp.tile([128, NCH, HW], F32)
                for oc in range(NCH):
                    psum_o = psA.tile([128, 2, 512], F32, tag="ps_big")
                    for half in range(2):
                        for ci in range(NCH):
                            nc.tensor.matmul(
                                psum_o[:, half, :],
                                lhsT=wo_bf[:, ci, oc * 128:(oc + 1) * 128],
                                rhs=oatt[:, ci, half * 512:(half + 1) * 512],
                                start=(ci == 0),
                                stop=(ci == NCH - 1 and not with_bo),
                            )
                        if with_bo:
                            nc.tensor.matmul(
                                psum_o[:, half, :],
                                lhsT=bo_bf[:, oc * 128:(oc + 1) * 128],
                                rhs=ones512[:], start=False, stop=True,
                            )
                    nc.vector.tensor_add(
                        out_sb[:, oc, :],
                        psum_o[:].rearrange("p a b -> p (a b)"),
                        x_sb[:, oc, :])

                nc.gpsimd.dma_start(out=_chunked(out_d[:, f, :]), in_=out_sb[:])

    nc.finalize()
    return nc


def _prep_in_maps(x, context, gamma, beta, wq, bq, wkv, bkv, wo, bo):
    f32 = lambda a: np.ascontiguousarray(np.asarray(a, dtype=np.float32))
    x, context = f32(x), f32(context)
    wqT = f32(np.asarray(wq, np.float32).T)
    wkvT = f32(np.asarray(wkv, np.float32).T)
    woT = f32(np.asarray(wo, np.float32).T)
    gammaT = f32(np.asarray(gamma, np.float32).reshape(NCH, 128).T)
    betaT = f32(np.asarray(beta, np.float32).reshape(NCH, 128).T)
    bqT = f32(np.asarray(bq, np.float32).reshape(NCH, 128).T)
    bkv_r = f32(np.asarray(bkv, np.float32).reshape(1, 2 * C))
    bo_r = f32(np.asarray(bo, np.float32).reshape(1, C))

    gmat = np.zeros((128, 8), np.float32)
    gmat[np.arange(128), np.arange(128) // CPG] = 1.0 / CPG
    emat = np.zeros((8, 128), np.float32)
    emat[np.arange(128) // CPG, np.arange(128)] = 1.0

    in_maps = []
    for core in range(NCORES):
        b, r = divmod(core, 4)
        xs = np.ascontiguousarray(x[b, :, r::4, :, :].reshape(C, FPC, HW))
        ctxT = np.ascontiguousarray(context[b].T)
        mask = np.zeros((FPC, 1, S), np.float32)
        for f in range(FPC):
            t = 4 * f + r
            lim = min(4 * (t + 1), S)
            mask[f, 0, lim:] = NEGINF
        in_maps.append(dict(
            x=xs, ctxT=ctxT, wqT=wqT, wkvT=wkvT, woT=woT,
            gammaT=gammaT, betaT=betaT, bqT=bqT, bkv=bkv_r, bo=bo_r,
            mask=mask, gmat=gmat, emat=emat,
        ))
    return in_maps


def kernel(x, context, gamma, beta, wq, bq, wkv, bkv, wo, bo,
           _trace=False, **_trace_kwargs):
    global LAST_RESULT
    with_bkv = bool(np.any(np.asarray(bkv)))
    with_bo = bool(np.any(np.asarray(bo)))
    key = (with_bkv, with_bo)
    if key not in _GRAPH_CACHE:
        _GRAPH_CACHE[key] = _build(*key)
    nc = _GRAPH_CACHE[key]

    in_maps = _prep_in_maps(x, context, gamma, beta, wq, bq, wkv, bkv, wo, bo)
    res = run_bass_kernel_spmd(nc, in_maps, core_ids=list(range(NCORES)),
                               trace=_trace, **_trace_kwargs)
    LAST_RESULT = res

    out = np.empty((B, C, T, H, W), np.float32)
    for core in range(NCORES):
        b, r = divmod(core, 4)
        out[b, :, r::4, :, :] = res.results[core]["out"].reshape(C, FPC, H, W)
    return out


# revision 7
# speedup vs baseline: 1.1274x; 1.1274x over previous
"""Trainium2 Bass kernel: CausalCrossAttention (GroupNorm + Q proj + block-causal
cross-attention over a small context + out proj + residual).

Sharding: 8 cores, each owns one (batch b, frame-residue r) pair:
  b = core // 4, r = core % 4, frames t = r + 4*f for f in 0..3.
GroupNorm normalizes each (b, t) frame independently over (16ch x H*W), and
attention key/value come from the (tiny) per-batch context, so every core's
work is fully local -- no collectives.  The block-causal mask is shipped as a
per-core additive bias tensor so all cores run the identical SPMD graph.

All heavy matmuls run in bf16 on the TensorEngine (f32 accumulation in PSUM);
GroupNorm statistics are computed in f32 (bn_stats/bn_aggr + tiny f32 matmuls
to fold 16-channel groups across partitions).  rsqrt(var+eps) is computed with
the bit-trick + 2 Newton steps entirely on the VectorEngine, so the
ScalarEngine uses only Copy/Identity/Exp -- one activation table set for the
whole kernel (table switches cost ~2.7us each).

The frame loop is software-pipelined: frame f+1's x-load and statistics are
emitted interleaved with frame f's attention so DVE/ACT/PE/DMA overlap.
"""

import numpy as np

import concourse.bass as bass
import concourse.bacc as bacc
import concourse.mybir as mybir
import concourse.tile as tile
from concourse.bass_utils import run_bass_kernel_spmd
from concourse.masks import make_identity

# Problem shape (fixed by the harness).
B, C, T, H, W = 2, 512, 16, 32, 32
HW = H * W            # 1024 query positions per frame
S, D = 64, 1024       # context length, context dim
G = 32                # groupnorm groups
CPG = C // G          # 16 channels per group
NCORES = 8
FPC = (B * T) // NCORES   # 4 frames per core
NCH = C // 128        # 4 channel chunks of 128
NDCH = D // 128       # 8 context-dim chunks
EPS = 1e-5
SCALE = float(C) ** -0.5
NEGINF = -1e9
# quake rsqrt seed magic, pre-adjusted for taking bits of 0.5*x instead of x
MAGIC_HALF = 0x5F3759DF - 0x00400000

F32 = mybir.dt.float32
BF16 = mybir.dt.bfloat16
I32 = mybir.dt.int32

Identity = mybir.ActivationFunctionType.Identity
Copy = mybir.ActivationFunctionType.Copy
Exp = mybir.ActivationFunctionType.Exp
Alu = mybir.AluOpType

LAST_RESULT = None        # BassKernelResults of the most recent run (for test.py)
_GRAPH_CACHE = {}


def _chunked(dram_ap):
    """[N*128, ...] dram AP -> [128, N, ...] with channel = n*128 + p."""
    return dram_ap.rearrange("(a p) w -> p a w", p=128)


def _build(with_bkv: bool, with_bo: bool) -> bass.Bass:
    nc = bacc.Bacc()

    x_d = nc.declare_dram_parameter("x", [C, FPC, HW], F32, isOutput=False)
    ctxT_d = nc.declare_dram_parameter("ctxT", [D, S], F32, isOutput=False)
    wqT_d = nc.declare_dram_parameter("wqT", [C, C], F32, isOutput=False)
    wkvT_d = nc.declare_dram_parameter("wkvT", [D, 2 * C], F32, isOutput=False)
    woT_d = nc.declare_dram_parameter("woT", [C, C], F32, isOutput=False)
    gammaT_d = nc.declare_dram_parameter("gammaT", [128, NCH], F32, isOutput=False)
    betaT_d = nc.declare_dram_parameter("betaT", [128, NCH], F32, isOutput=False)
    bqT_d = nc.declare_dram_parameter("bqT", [128, NCH], F32, isOutput=False)
    bkv_d = nc.declare_dram_parameter("bkv", [1, 2 * C], F32, isOutput=False)
    bo_d = nc.declare_dram_parameter("bo", [1, C], F32, isOutput=False)
    mask_d = nc.declare_dram_parameter("mask", [FPC, 1, S], F32, isOutput=False)
    gmat_d = nc.declare_dram_parameter("gmat", [128, 8], F32, isOutput=False)
    emat_d = nc.declare_dram_parameter("emat", [8, 128], F32, isOutput=False)
    out_d = nc.declare_dram_parameter("out", [C, FPC, HW], F32, isOutput=True)

    with tile.TileContext(nc) as tc:
        with (
            tc.tile_pool(name="weights", bufs=1) as wp,
            tc.tile_pool(name="stage", bufs=2) as stage,
            tc.tile_pool(name="xp", bufs=2) as xp,
            tc.tile_pool(name="hp", bufs=2) as hp,
            tc.tile_pool(name="qp", bufs=2) as qp,
            tc.tile_pool(name="op", bufs=2) as op_,
            tc.tile_pool(name="rp", bufs=2) as rp,
            tc.tile_pool(name="small", bufs=2) as small,
            tc.tile_pool(name="psA", bufs=3, space="PSUM") as psA,
            tc.tile_pool(name="psB", bufs=2, space="PSUM") as psB,
        ):
            # ---------------- constants ----------------
            gammaT_sb = wp.tile([128, NCH], F32)
            betaT_sb = wp.tile([128, NCH], F32)
            bqT_sb = wp.tile([128, NCH], F32)
            gmat_sb = wp.tile([128, 8], F32)
            emat_sb = wp.tile([8, 128], F32)
            mask_sb = wp.tile([128, FPC, 1, S], F32)
            identity = wp.tile([128, 128], BF16)
            magic_sb = wp.tile([8, NCH], I32)

            nc.sync.dma_start(out=gammaT_sb[:], in_=gammaT_d[:, :])
            nc.sync.dma_start(out=betaT_sb[:], in_=betaT_d[:, :])
            nc.sync.dma_start(out=bqT_sb[:], in_=bqT_d[:, :])
            nc.sync.dma_start(out=gmat_sb[:], in_=gmat_d[:, :])
            nc.sync.dma_start(out=emat_sb[:], in_=emat_d[:, :])
            m_ap = mask_d[:, :, :]
            nc.sync.dma_start(
                out=mask_sb[:],
                in_=bass.AP(tensor=m_ap.tensor, offset=m_ap.offset,
                            ap=[[0, 128]] + list(m_ap.ap)),
            )
            make_identity(nc, identity[:])
            nc.gpsimd.memset(magic_sb[:], MAGIC_HALF)

            # ---------------- weights: DMA f32 stage -> bf16 cast ----------------
            wq_bf = wp.tile([128, NCH, C], BF16)
            wkv_bf = wp.tile([128, NDCH, 2 * C], BF16)
            wo_bf = wp.tile([128, NCH, C], BF16)
            ctx_bf = wp.tile([128, NDCH, S], BF16)

            def load_cast(dst_slice, src_ap, eng):
                st = stage.tile([128, 1024], F32, tag="stage")
                stv = st[:].rearrange("p (a w) -> p a w", w=src_ap.shape[-1])
                n = src_ap.shape[1]
                nc.sync.dma_start(out=stv[:, :n, :], in_=src_ap)
                if eng == "v":
                    nc.vector.tensor_copy(out=dst_slice, in_=stv[:, :n, :])
                else:
                    nc.scalar.activation(out=dst_slice, in_=stv[:, :n, :], func=Copy)

            wqT_c = _chunked(wqT_d[:, :])      # [128, 4, 512]
            woT_c = _chunked(woT_d[:, :])      # [128, 4, 512]
            wkvT_c = _chunked(wkvT_d[:, :])    # [128, 8, 1024]
            ctxT_c = _chunked(ctxT_d[:, :])    # [128, 8, 64]

            load_cast(ctx_bf[:], ctxT_c, "v")
            for i in range(2):
                load_cast(wq_bf[:, 2 * i:2 * i + 2, :], wqT_c[:, 2 * i:2 * i + 2, :],
                          "v" if i == 0 else "s")
            for i in range(NDCH):
                load_cast(wkv_bf[:, i:i + 1, :], wkvT_c[:, i:i + 1, :],
                          "v" if i % 2 == 0 else "s")
            for i in range(2):
                load_cast(wo_bf[:, 2 * i:2 * i + 2, :], woT_c[:, 2 * i:2 * i + 2, :],
                          "v" if i == 0 else "s")

            if with_bkv:
                ones64 = wp.tile([1, S], BF16)
                nc.vector.memset(ones64[:], 1.0)
                stb = small.tile([1, 2 * C], F32)
                nc.sync.dma_start(out=stb[:], in_=bkv_d[:, :])
                bkv_bf = wp.tile([1, 2 * C], BF16)
                nc.vector.tensor_copy(out=bkv_bf[:], in_=stb[:])
            if with_bo:
                ones512 = wp.tile([1, 512], BF16)
                nc.vector.memset(ones512[:], 1.0)
                sbo = small.tile([1, C], F32)
                nc.sync.dma_start(out=sbo[:], in_=bo_d[:, :])
                bo_bf = wp.tile([1, C], BF16)
                nc.vector.tensor_copy(out=bo_bf[:], in_=sbo[:])

            # ---------------- kv projection (once per core) ----------------
            # kv[s, o] = sum_d ctx[s, d] * wkv[o, d] (+ bkv);  k = kv[:, :C], v = kv[:, C:]
            kT_sb = wp.tile([128, NCH, S], BF16)   # k transposed: [c, s]
            v_sb = wp.tile([64, C], BF16)          # v: [s, c]
            for half in range(2):
                psum_kv = psB.tile([64, 512], F32, tag="ps_small")
                for dci in range(NDCH):
                    nc.tensor.matmul(
                        psum_kv[:],
                        lhsT=ctx_bf[:, dci, :],
                        rhs=wkv_bf[:, dci, half * 512:(half + 1) * 512],
                        start=(dci == 0),
                        stop=(dci == NDCH - 1 and not with_bkv),
                    )
                if with_bkv:
                    nc.tensor.matmul(
                        psum_kv[:], lhsT=ones64[:],
                        rhs=bkv_bf[:, half * 512:(half + 1) * 512],
                        start=False, stop=True,
                    )
                if half == 0:
                    k_sb = small.tile([64, 512], BF16)
                    nc.scalar.activation(out=k_sb[:], in_=psum_kv[:], func=Copy)
                    psum_kT = psB.tile([128, NCH, S], BF16, tag="ps_small")
                    for ci in range(NCH):
                        nc.tensor.transpose(
                            psum_kT[:, ci, :],
                            k_sb[:, ci * 128:(ci + 1) * 128],
                            identity[:64, :64],
                        )
                    nc.scalar.activation(out=kT_sb[:], in_=psum_kT[:], func=Copy)
                else:
                    nc.scalar.activation(out=v_sb[:], in_=psum_kv[:], func=Copy)

            # ---------------- software-pipelined frame loop ----------------
            x_tiles = [None] * FPC
            ab_tiles = [None] * FPC   # (a_sb, b_sb) per frame
            mv_tiles = [None] * FPC

            def emit_x_load(f):
                x_sb = xp.tile([128, NCH, HW], F32)
                nc.sync.dma_start(out=x_sb[:], in_=_chunked(x_d[:, f, :]))
                x_tiles[f] = x_sb

            def emit_stats_dve(f):
                """Per-channel mean / E[x^2] via bn_stats (DVE only)."""
                x_sb = x_tiles[f]
                st6 = small.tile([128, NCH, 2, 6], F32)
                mv = small.tile([128, NCH, 2], F32)
                for ci in range(NCH):
                    xv = x_sb[:, ci, :].rearrange("p (a b) -> p a b", a=2)
                    for k2 in range(2):
                        nc.vector.bn_stats(out=st6[:, ci, k2, :], in_=xv[:, k2, :])
                    nc.vector.bn_aggr(out=mv[:, ci, :], in_=st6[:, ci, :, :])
                msq = small.tile([128, NCH], F32)
                nc.vector.tensor_mul(msq[:], mv[:, :, 0], mv[:, :, 0])
                nc.vector.tensor_add(mv[:, :, 1], mv[:, :, 1], msq[:])
                mv_tiles[f] = mv

            def emit_stats_fold(f):
                """Group-fold matmul: [128, 8] -> [8, 8] per-group (mean, E2)."""
                psum_g = psB.tile([8, 8], F32, tag="ps_small")
                nc.tensor.matmul(
                    psum_g[:], lhsT=gmat_sb[:],
                    rhs=mv_tiles[f][:].rearrange("p a b -> p (a b)"),
                    start=True, stop=True,
                )
                return psum_g

            def emit_stats_finish(f, psum_g):
                """rsqrt via bit trick + 2 Newton steps (DVE), expand to channels,
                produce per-channel affine (a, b)."""
                gs = small.tile([8, NCH, 2], F32)
                nc.vector.tensor_copy(
                    out=gs[:], in_=psum_g[:].rearrange("p (a b) -> p a b", a=NCH))
                gsq = small.tile([8, NCH], F32)
                nc.vector.tensor_mul(gsq[:], gs[:, :, 0], gs[:, :, 0])
                hx = small.tile([8, NCH], F32)
                # hx = 0.5 * (E2 - mean^2 + eps) = 0.5 * (var + eps)
                nc.vector.tensor_sub(hx[:], gs[:, :, 1], gsq[:])
                nc.vector.tensor_scalar(
                    out=hx[:], in0=hx[:], scalar1=EPS, scalar2=0.5,
                    op0=Alu.add, op1=Alu.mult)
                ya = small.tile([8, NCH], F32)
                yb = small.tile([8, NCH], F32)
                sh = small.tile([8, NCH], I32)
                nc.vector.tensor_scalar(
                    out=sh[:], in0=hx[:].bitcast(I32), scalar1=1, scalar2=None,
                    op0=Alu.arith_shift_right)
                nc.vector.tensor_sub(ya[:].bitcast(I32), magic_sb[:], sh[:])
                u = small.tile([8, NCH], F32)
                cur, nxt = ya, yb
                for _ in range(2):
                    nc.vector.tensor_mul(u[:], cur[:], cur[:])
                    nc.vector.tensor_mul(u[:], u[:], hx[:])
                    # (hx*y^2 - 1.5) * y = -y*(1.5 - hx*y^2); sign cancels over 2 iters
                    nc.vector.scalar_tensor_tensor(
                        out=nxt[:], in0=u[:], scalar=1.5, in1=cur[:],
                        op0=Alu.subtract, op1=Alu.mult)
                    cur, nxt = nxt, cur
                # write rs into gs[:, :, 1]
                nc.vector.tensor_copy(out=gs[:, :, 1], in_=cur[:])
                # expand group stats back to channels: [8, 8] -> [128, (4, 2)]
                psum_e = psB.tile([128, NCH, 2], F32, tag="ps_small")
                nc.tensor.matmul(
                    psum_e[:].rearrange("p a b -> p (a b)"),
                    lhsT=emat_sb[:], rhs=gs[:].rearrange("p a b -> p (a b)"),
                    start=True, stop=True,
                )
                a_sb = small.tile([128, NCH], F32)
                t_sb = small.tile([128, NCH], F32)
                b_sb = small.tile([128, NCH], F32)
                nc.vector.tensor_mul(a_sb[:], psum_e[:, :, 1], gammaT_sb[:])
                nc.vector.tensor_mul(t_sb[:], psum_e[:, :, 0], a_sb[:])
                nc.vector.tensor_sub(b_sb[:], betaT_sb[:], t_sb[:])
                ab_tiles[f] = (a_sb, b_sb)

            # prologue: frame 0 statistics
            emit_x_load(0)
            emit_stats_dve(0)
            pg = emit_stats_fold(0)
            emit_stats_finish(0, pg)

            for f in range(FPC):
                x_sb = x_tiles[f]
                a_sb, b_sb = ab_tiles[f]

                # next frame's x-load + DVE statistics, overlapped with frame f
                if f + 1 < FPC:
                    emit_x_load(f + 1)
                    emit_stats_dve(f + 1)

                # normalize: h = a*x + b (bf16)
                h_sb = hp.tile([128, NCH, HW], BF16)
                for ci in range(NCH):
                    nc.scalar.activation(
                        out=h_sb[:, ci, :], in_=x_sb[:, ci, :], func=Identity,
                        bias=b_sb[:, ci:ci + 1], scale=a_sb[:, ci:ci + 1])

                # q projection: q[oc, p] = sum_c wq[oc, c] h[c, p] + bq
                q_sb = qp.tile([128, NCH, HW], BF16)
                for oc in range(NCH):
                    psum_q = psA.tile([128, 2, 512], F32, tag="ps_big")
                    for half in range(2):
                        for ci in range(NCH):
                            nc.tensor.matmul(
                                psum_q[:, half, :],
                                lhsT=wq_bf[:, ci, oc * 128:(oc + 1) * 128],
                                rhs=h_sb[:, ci, half * 512:(half + 1) * 512],
                                start=(ci == 0), stop=(ci == NCH - 1),
                            )
                    nc.scalar.activation(
                        out=q_sb[:, oc, :],
                        in_=psum_q[:].rearrange("p a b -> p (a b)"),
                        func=Identity, bias=bqT_sb[:, oc:oc + 1], scale=1.0)

                # scores: [q(8x128), s]
                psum_s = psB.tile([128, 8, S], F32, tag="ps_small")
                for j in range(8):
                    for ci in range(NCH):
                        nc.tensor.matmul(
                            psum_s[:, j, :],
                            lhsT=q_sb[:, ci, j * 128:(j + 1) * 128],
                            rhs=kT_sb[:, ci, :],
                            start=(ci == 0), stop=(ci == NCH - 1),
                        )

                # next frame's group-fold matmul sits in the QK->softmax gap
                psum_g_next = emit_stats_fold(f + 1) if f + 1 < FPC else None

                # softmax over s (no max-subtraction: |scale*scores| is small)
                sc_sb = small.tile([128, 8, S], F32)
                nc.vector.tensor_add(
                    sc_sb[:], psum_s[:],
                    mask_sb[:, f, :, :].to_broadcast((128, 8, S)))
                p_sb = small.tile([128, 8, S], F32)
                nc.scalar.activation(out=p_sb[:], in_=sc_sb[:], func=Exp, scale=SCALE)
                l8 = small.tile([128, 8, 1], F32)
                nc.vector.reduce_sum(l8[:], p_sb[:], axis=mybir.AxisListType.X)
                linv = small.tile([128, 8, 1], F32)
                nc.vector.reciprocal(linv[:], l8[:])
                p_bf = small.tile([128, 8, S], BF16)
                nc.vector.tensor_mul(p_bf[:], p_sb[:], linv[:].to_broadcast((128, 8, S)))

                # transpose weights to [s, q] for the PV matmul
                psum_wT = psB.tile([64, 8, 128], BF16, tag="ps_small")
                for j in range(8):
                    nc.tensor.transpose(psum_wT[:, j, :], p_bf[:, j, :], identity[:])
                wT_sb = small.tile([64, 8, 128], BF16)
                nc.scalar.activation(out=wT_sb[:], in_=psum_wT[:], func=Copy)
                wT_flat = wT_sb[:].rearrange("p a b -> p (a b)")  # [64, 1024]

                # next frame's rsqrt + expand, in the transpose->PV gap
                if psum_g_next is not None:
                    emit_stats_finish(f + 1, psum_g_next)

                # attention output (transposed): o[c, q] = sum_s v[s, c] w[q, s]
                oatt = op_.tile([128, NCH, HW], BF16)
                for ci in range(NCH):
                    psum_pv = psA.tile([128, 2, 512], F32, tag="ps_big")
                    for half in range(2):
                        nc.tensor.matmul(
                            psum_pv[:, half, :],
                            lhsT=v_sb[:, ci * 128:(ci + 1) * 128],
                            rhs=wT_flat[:, half * 512:(half + 1) * 512],
                            start=True, stop=True,
                        )
                    nc.scalar.activation(
                        out=oatt[:, ci, :],
                        in_=psum_pv[:].rearrange("p a b -> p (a b)"), func=Copy)

                # output projection + residual
                out_sb = rp.tile([128, NCH, HW], F32)
                for oc in range(NCH):
                    psum_o = psA.tile([128, 2, 512], F32, tag="ps_big")
                    for half in range(2):
                        for ci in range(NCH):
                            nc.tensor.matmul(
                                psum_o[:, half, :],
                                lhsT=wo_bf[:, ci, oc * 128:(oc + 1) * 128],
                                rhs=oatt[:, ci, half * 512:(half + 1) * 512],
                                start=(ci == 0),
                                stop=(ci == NCH - 1 and not with_bo),
                            )
                        if with_bo:
                            nc.tensor.matmul(
                                psum_o[:, half, :],
                                lhsT=bo_bf[:, oc * 128:(oc + 1) * 128],
                                rhs=ones512[:], start=False, stop=True,
                            )
                    nc.vector.tensor_add(
                        out_sb[:, oc, :],
                        psum_o[:].rearrange("p a b -> p (a b)"),
                        x_sb[:, oc, :])

                nc.sync.dma_start(out=_chunked(out_d[:, f, :]), in_=out_sb[:])

    nc.finalize()
    return nc


def _prep_in_maps(x, context, gamma, beta, wq, bq, wkv, bkv, wo, bo):
    f32 = lambda a: np.ascontiguousarray(np.asarray(a, dtype=np.float32))
    x, context = f32(x), f32(context)
    wqT = f32(np.asarray(wq, np.float32).T)
    wkvT = f32(np.asarray(wkv, np.float32).T)
    woT = f32(np.asarray(wo, np.float32).T)
    gammaT = f32(np.asarray(gamma, np.float32).reshape(NCH, 128).T)
    betaT = f32(np.asarray(beta, np.float32).reshape(NCH, 128).T)
    bqT = f32(np.asarray(bq, np.float32).reshape(NCH, 128).T)
    bkv_r = f32(np.asarray(bkv, np.float32).reshape(1, 2 * C))
    bo_r = f32(np.asarray(bo, np.float32).reshape(1, C))

    gmat = np.zeros((128, 8), np.float32)
    gmat[np.arange(128), np.arange(128) // CPG] = 1.0 / CPG
    emat = np.zeros((8, 128), np.float32)
    emat[np.arange(128) // CPG, np.arange(128)] = 1.0

    in_maps = []
    for core in range(NCORES):
        b, r = divmod(core, 4)
        xs = np.ascontiguousarray(x[b, :, r::4, :, :].reshape(C, FPC, HW))
        ctxT = np.ascontiguousarray(context[b].T)
        mask = np.zeros((FPC, 1, S), np.float32)
        for f in range(FPC):
            t = 4 * f + r
            lim = min(4 * (t + 1), S)
            mask[f, 0, lim:] = NEGINF
        in_maps.append(dict(
            x=xs, ctxT=ctxT, wqT=wqT, wkvT=wkvT, woT=woT,
            gammaT=gammaT, betaT=betaT, bqT=bqT, bkv=bkv_r, bo=bo_r,
            mask=mask, gmat=gmat, emat=emat,
        ))
    return in_maps


def kernel(x, context, gamma, beta, wq, bq, wkv, bkv, wo, bo,
           _trace=False, **_trace_kwargs):
    global LAST_RESULT
    with_bkv = bool(np.any(np.asarray(bkv)))
    with_bo = bool(np.any(np.asarray(bo)))
    key = (with_bkv, with_bo)
    if key not in _GRAPH_CACHE:
        _GRAPH_CACHE[key] = _build(*key)
    nc = _GRAPH_CACHE[key]

    in_maps = _prep_in_maps(x, context, gamma, beta, wq, bq, wkv, bkv, wo, bo)
    res = run_bass_kernel_spmd(nc, in_maps, core_ids=list(range(NCORES)),
                               trace=_trace, **_trace_kwargs)
    LAST_RESULT = res

    out = np.empty((B, C, T, H, W), np.float32)
    for core in range(NCORES):
        b, r = divmod(core, 4)
        out[b, :, r::4, :, :] = res.results[core]["out"].reshape(C, FPC, H, W)
    return out


# revision 9
# speedup vs baseline: 1.6543x; 1.4673x over previous
"""Trainium2 Bass kernel: CausalCrossAttention (GroupNorm + Q proj + block-causal
cross-attention over a small context + out proj + residual).

Sharding: 8 cores, each owns one (batch b, frame-residue r) pair:
  b = core // 4, r = core % 4, frames t = r + 4*f for f in 0..3.
GroupNorm normalizes each (b, t) frame independently over (16ch x H*W), and
attention key/value come from the (tiny) per-batch context, so every core's
work is fully local -- no collectives.  The block-causal mask is shipped as a
per-core additive bias and applied inside PSUM via a rank-1 (K=1) matmul, so
all cores run the identical SPMD graph.

Key algebraic fusion (exact, by associativity): with S=64 << H*W=1024 the
projections fold into the context side:
    scores = (Wq h)^T k  = h^T (Wq^T k)  = h^T kq,      kq = Wq^T k   [C, S]
    out    = Wo (v^T w)  = (Wo v^T) w    = vo^T w,      vo = v Wo^T   [S, C]
kq / vo / k / v are tiny per-core constants computed once from the context,
so the per-frame work is one [C x S] contraction + softmax + one [S x C]
contraction -- ~9x fewer FLOPs than materializing q and the o-projection.

All heavy matmuls run in bf16 (f32 PSUM accumulation); GroupNorm statistics in
f32 (bn_stats/bn_aggr + tiny f32 matmuls to fold/expand 16-channel groups
across partitions).  rsqrt(var+eps) is computed with the bit-trick + 2 Newton
steps entirely on the VectorEngine so the ScalarEngine only ever needs one
activation table set (Copy/Identity/Exp).  The frame loop is software
pipelined: frame f+1's x-load and statistics interleave with frame f.
"""

import numpy as np

import concourse.bass as bass
import concourse.bacc as bacc
import concourse.mybir as mybir
import concourse.tile as tile
from concourse.bass_utils import run_bass_kernel_spmd
from concourse.masks import make_identity

# Problem shape (fixed by the harness).
B, C, T, H, W = 2, 512, 16, 32, 32
HW = H * W            # 1024 query positions per frame
S, D = 64, 1024       # context length, context dim
G = 32                # groupnorm groups
CPG = C // G          # 16 channels per group
NCORES = 8
FPC = (B * T) // NCORES   # 4 frames per core
NCH = C // 128        # 4 channel chunks of 128
NDCH = D // 128       # 8 context-dim chunks
EPS = 1e-5
SCALE = float(C) ** -0.5
NEGINF = -1e9
# quake rsqrt seed magic, pre-adjusted for taking bits of 0.5*x instead of x
MAGIC_HALF = 0x5F3759DF - 0x00400000

F32 = mybir.dt.float32
BF16 = mybir.dt.bfloat16
I32 = mybir.dt.int32

Identity = mybir.ActivationFunctionType.Identity
Copy = mybir.ActivationFunctionType.Copy
Exp = mybir.ActivationFunctionType.Exp
Alu = mybir.AluOpType

LAST_RESULT = None        # BassKernelResults of the most recent run (for test.py)
_GRAPH_CACHE = {}


def _chunked(dram_ap):
    """[N*128, ...] dram AP -> [128, N, ...] with channel = n*128 + p."""
    return dram_ap.rearrange("(a p) w -> p a w", p=128)


def _build(with_bq: bool, with_bkv: bool, with_bo: bool) -> bass.Bass:
    nc = bacc.Bacc()

    x_d = nc.declare_dram_parameter("x", [C, FPC, HW], F32, isOutput=False)
    ctxT_d = nc.declare_dram_parameter("ctxT", [D, S], F32, isOutput=False)
    wqT_d = nc.declare_dram_parameter("wqT", [C, C], F32, isOutput=False)
    wkvT_d = nc.declare_dram_parameter("wkvT", [D, 2 * C], F32, isOutput=False)
    woT_d = nc.declare_dram_parameter("woT", [C, C], F32, isOutput=False)
    gammaT_d = nc.declare_dram_parameter("gammaT", [128, NCH], F32, isOutput=False)
    betaT_d = nc.declare_dram_parameter("betaT", [128, NCH], F32, isOutput=False)
    bqT_d = nc.declare_dram_parameter("bqT", [128, NCH], F32, isOutput=False)
    bkv_d = nc.declare_dram_parameter("bkv", [1, 2 * C], F32, isOutput=False)
    bo_d = nc.declare_dram_parameter("bo", [1, C], F32, isOutput=False)
    mask_d = nc.declare_dram_parameter("mask", [1, FPC, S], F32, isOutput=False)
    gmat_d = nc.declare_dram_parameter("gmat", [128, 8], F32, isOutput=False)
    emat_d = nc.declare_dram_parameter("emat", [8, 128], F32, isOutput=False)
    out_d = nc.declare_dram_parameter("out", [C, FPC, HW], F32, isOutput=True)

    with tile.TileContext(nc) as tc:
        with (
            tc.tile_pool(name="consts", bufs=1) as wp,
            tc.tile_pool(name="wtmp", bufs=1) as wtmp,
            tc.tile_pool(name="stage", bufs=4) as stage,
            tc.tile_pool(name="xp", bufs=3) as xp,
            tc.tile_pool(name="hp", bufs=2) as hp,
            tc.tile_pool(name="small", bufs=2) as small,
            tc.tile_pool(name="psO", bufs=2, space="PSUM") as psO,
            tc.tile_pool(name="psB", bufs=4, space="PSUM") as psB,
        ):
            # ---------------- constants ----------------
            gammaT_sb = wp.tile([128, NCH], F32)
            betaT_sb = wp.tile([128, NCH], F32)
            bqT_sb = wp.tile([128, NCH], F32)
            gmat_sb = wp.tile([128, 8], F32)
            emat_sb = wp.tile([8, 128], F32)
            mask_f32 = wp.tile([1, FPC, S], F32)
            mask_bf = wp.tile([1, FPC, S], BF16)
            ones_col = wp.tile([1, 128], BF16)
            identity = wp.tile([128, 128], BF16)
            magic_sb = wp.tile([8, NCH], I32)

            nc.sync.dma_start(out=gammaT_sb[:], in_=gammaT_d[:, :])
            nc.sync.dma_start(out=betaT_sb[:], in_=betaT_d[:, :])
            nc.sync.dma_start(out=bqT_sb[:], in_=bqT_d[:, :])
            nc.sync.dma_start(out=gmat_sb[:], in_=gmat_d[:, :])
            nc.sync.dma_start(out=emat_sb[:], in_=emat_d[:, :])
            nc.sync.dma_start(out=mask_f32[:], in_=mask_d[:, :, :])
            make_identity(nc, identity[:])
            nc.gpsimd.memset(magic_sb[:], MAGIC_HALF)
            nc.gpsimd.memset(ones_col[:], 1.0)

            # ---------------- frame 0 x-load + DVE statistics (early) -------------
            x_tiles = [None] * FPC
            ab_tiles = [None] * FPC
            mv_tiles = [None] * FPC

            def emit_x_load(f):
                x_sb = xp.tile([128, NCH, HW], F32)
                nc.sync.dma_start(out=x_sb[:], in_=_chunked(x_d[:, f, :]))
                x_tiles[f] = x_sb

            def emit_stats_dve(f):
                """Per-channel mean / E[x^2] via bn_stats (DVE only)."""
                x_sb = x_tiles[f]
                st6 = small.tile([128, NCH, 2, 6], F32)
                mv = small.tile([128, NCH, 2], F32)
                for ci in range(NCH):
                    xv = x_sb[:, ci, :].rearrange("p (a b) -> p a b", a=2)
                    for k2 in range(2):
                        nc.vector.bn_stats(out=st6[:, ci, k2, :], in_=xv[:, k2, :])
                    nc.vector.bn_aggr(out=mv[:, ci, :], in_=st6[:, ci, :, :])
                msq = small.tile([128, NCH], F32)
                nc.vector.tensor_mul(msq[:], mv[:, :, 0], mv[:, :, 0])
                nc.vector.tensor_add(mv[:, :, 1], mv[:, :, 1], msq[:])
                mv_tiles[f] = mv

            def emit_stats_fold(f):
                psum_g = psB.tile([8, 8], F32, tag="ps_small")
                nc.tensor.matmul(
                    psum_g[:], lhsT=gmat_sb[:],
                    rhs=mv_tiles[f][:].rearrange("p a b -> p (a b)"),
                    start=True, stop=True,
                )
                return psum_g

            def emit_stats_finish(f, psum_g):
                """rsqrt via bit trick + 2 Newton steps (DVE), expand to channels,
                produce per-channel affine (a, b)."""
                gs = small.tile([8, NCH, 2], F32)
                nc.vector.tensor_copy(
                    out=gs[:], in_=psum_g[:].rearrange("p (a b) -> p a b", a=NCH))
                gsq = small.tile([8, NCH], F32)
                nc.vector.tensor_mul(gsq[:], gs[:, :, 0], gs[:, :, 0])
                hx = small.tile([8, NCH], F32)
                nc.vector.tensor_sub(hx[:], gs[:, :, 1], gsq[:])
                nc.vector.tensor_scalar(
                    out=hx[:], in0=hx[:], scalar1=EPS, scalar2=0.5,
                    op0=Alu.add, op1=Alu.mult)
                ya = small.tile([8, NCH], F32)
                yb = small.tile([8, NCH], F32)
                sh = small.tile([8, NCH], I32)
                nc.vector.tensor_scalar(
                    out=sh[:], in0=hx[:].bitcast(I32), scalar1=1, scalar2=None,
                    op0=Alu.arith_shift_right)
                nc.vector.tensor_sub(ya[:].bitcast(I32), magic_sb[:], sh[:])
                u = small.tile([8, NCH], F32)
                cur, nxt = ya, yb
                for _ in range(2):
                    nc.vector.tensor_mul(u[:], cur[:], cur[:])
                    nc.vector.tensor_mul(u[:], u[:], hx[:])
                    nc.vector.scalar_tensor_tensor(
                        out=nxt[:], in0=u[:], scalar=1.5, in1=cur[:],
                        op0=Alu.subtract, op1=Alu.mult)
                    cur, nxt = nxt, cur
                nc.vector.tensor_copy(out=gs[:, :, 1], in_=cur[:])
                psum_e = psB.tile([128, NCH, 2], F32, tag="ps_small")
                nc.tensor.matmul(
                    psum_e[:].rearrange("p a b -> p (a b)"),
                    lhsT=emat_sb[:], rhs=gs[:].rearrange("p a b -> p (a b)"),
                    start=True, stop=True,
                )
                a_sb = small.tile([128, NCH], F32)
                t_sb = small.tile([128, NCH], F32)
                b_sb = small.tile([128, NCH], F32)
                nc.vector.tensor_mul(a_sb[:], psum_e[:, :, 1], gammaT_sb[:])
                nc.vector.tensor_mul(t_sb[:], psum_e[:, :, 0], a_sb[:])
                nc.vector.tensor_sub(b_sb[:], betaT_sb[:], t_sb[:])
                ab_tiles[f] = (a_sb, b_sb)

            emit_x_load(0)
            emit_stats_dve(0)

            # ---------------- weights: DMA f32 stage -> bf16 cast ----------------
            wq_bf = wtmp.tile([128, NCH, C], BF16)
            wkv_bf = wtmp.tile([128, NDCH, 2 * C], BF16)
            wo_bf = wtmp.tile([128, NCH, C], BF16)
            ctx_bf = wtmp.tile([128, NDCH, S], BF16)

            def load_cast(dst_slice, src_ap, eng):
                st = stage.tile([128, 1024], F32, tag="stage")
                stv = st[:].rearrange("p (a w) -> p a w", w=src_ap.shape[-1])
                n = src_ap.shape[1]
                nc.sync.dma_start(out=stv[:, :n, :], in_=src_ap)
                if eng == "v":
                    nc.vector.tensor_copy(out=dst_slice, in_=stv[:, :n, :])
                else:
                    nc.scalar.activation(out=dst_slice, in_=stv[:, :n, :], func=Copy)

            wqT_c = _chunked(wqT_d[:, :])      # [128, 4, 512]
            woT_c = _chunked(woT_d[:, :])      # [128, 4, 512]
            wkvT_c = _chunked(wkvT_d[:, :])    # [128, 8, 1024]
            ctxT_c = _chunked(ctxT_d[:, :])    # [128, 8, 64]

            load_cast(ctx_bf[:], ctxT_c, "s")
            for i in range(2):
                load_cast(wq_bf[:, 2 * i:2 * i + 2, :], wqT_c[:, 2 * i:2 * i + 2, :],
                          "v" if i == 0 else "s")
            for i in range(NDCH):
                load_cast(wkv_bf[:, i:i + 1, :], wkvT_c[:, i:i + 1, :],
                          "v" if i % 2 == 0 else "s")
            for i in range(2):
                load_cast(wo_bf[:, 2 * i:2 * i + 2, :], woT_c[:, 2 * i:2 * i + 2, :],
                          "v" if i == 0 else "s")

            nc.vector.tensor_copy(out=mask_bf[:], in_=mask_f32[:])
            if with_bkv:
                ones64 = wp.tile([1, S], BF16)
                nc.vector.memset(ones64[:], 1.0)
                stb = small.tile([1, 2 * C], F32)
                nc.sync.dma_start(out=stb[:], in_=bkv_d[:, :])
                bkv_bf = wp.tile([1, 2 * C], BF16)
                nc.vector.tensor_copy(out=bkv_bf[:], in_=stb[:])
            if with_bo:
                ones512 = wp.tile([1, 512], BF16)
                nc.vector.memset(ones512[:], 1.0)
                sbo = small.tile([1, C], F32)
                nc.sync.dma_start(out=sbo[:], in_=bo_d[:, :])
                bo_bf = wp.tile([1, C], BF16)
                nc.vector.tensor_copy(out=bo_bf[:], in_=sbo[:])

            # ---------------- context-side constants: k, v, kq, vo ---------------
            # kv[s, o] = sum_d ctx[s, d] wkv[o, d] (+ bkv); k = kv[:, :C], v = kv[:, C:]
            kT_sb = small.tile([128, NCH, S], BF16)   # k^T: [c, s]
            vT_sb = small.tile([128, NCH, S], BF16)   # v^T: [c, s]
            for half in range(2):
                psum_kv = psB.tile([64, 512], F32, tag="ps_small")
                for dci in range(NDCH):
                    nc.tensor.matmul(
                        psum_kv[:],
                        lhsT=ctx_bf[:, dci, :],
                        rhs=wkv_bf[:, dci, half * 512:(half + 1) * 512],
                        start=(dci == 0),
                        stop=(dci == NDCH - 1 and not with_bkv),
                    )
                if with_bkv:
                    nc.tensor.matmul(
                        psum_kv[:], lhsT=ones64[:],
                        rhs=bkv_bf[:, half * 512:(half + 1) * 512],
                        start=False, stop=True,
                    )
                kv_sb = small.tile([64, 512], BF16)
                nc.scalar.activation(out=kv_sb[:], in_=psum_kv[:], func=Copy)
                psum_t = psB.tile([128, NCH, S], BF16, tag="ps_small")
                for ci in range(NCH):
                    nc.tensor.transpose(
                        psum_t[:, ci, :],
                        kv_sb[:, ci * 128:(ci + 1) * 128],
                        identity[:64, :64],
                    )
                dst = kT_sb if half == 0 else vT_sb
                nc.scalar.activation(out=dst[:], in_=psum_t[:], func=Copy)

            # scores need m[c, s] = sum_c' wq[c', c] k[s, c']  (contraction over
            # Wq's OUTPUT index), so the stationary operand is wq in NATURAL
            # layout: lhsT[K=c'-chunk, M=c-tile] = wq[c', c].  The "wqT" dram
            # param therefore carries wq UNtransposed (see _prep_in_maps).
            kqT_sb = wp.tile([128, NCH, S], BF16)
            psum_kq = psB.tile([128, NCH, S], F32, tag="ps_small")
            for co in range(NCH):
                for ci in range(NCH):
                    nc.tensor.matmul(
                        psum_kq[:, co, :],
                        lhsT=wq_bf[:, ci, co * 128:(co + 1) * 128],
                        rhs=kT_sb[:, ci, :],
                        start=(ci == 0), stop=(ci == NCH - 1),
                    )
            nc.scalar.activation(out=kqT_sb[:], in_=psum_kq[:], func=Copy)

            # vo[s, oc] = sum_c v[s, c] wo[oc, c] = sum_c vT[c, s]^T woT[c, oc]
            vo_bf = wp.tile([64, C], BF16)
            psum_vo = psB.tile([64, 512], F32, tag="ps_small")
            for ci in range(NCH):
                nc.tensor.matmul(
                    psum_vo[:],
                    lhsT=vT_sb[:, ci, :],
                    rhs=wo_bf[:, ci, :],
                    start=(ci == 0), stop=(ci == NCH - 1),
                )
            nc.scalar.activation(out=vo_bf[:], in_=psum_vo[:], func=Copy)

            # bqk[s] = sum_c bq[c] k[s, c]: fold q-bias into the mask row
            if with_bq:
                psum_bq = psB.tile([1, S], F32, tag="ps_small")
                for ci in range(NCH):
                    nc.tensor.matmul(
                        psum_bq[:],
                        lhsT=bqT_sb[:, ci:ci + 1],
                        rhs=kT_sb[:, ci, :],
                        start=(ci == 0), stop=(ci == NCH - 1),
                    )
                for f in range(FPC):
                    nc.vector.tensor_add(mask_bf[:, f, :], mask_f32[:, f, :],
                                         psum_bq[:])

            # frame 0 statistics fold/finish (tiny PE + DVE)
            pg0 = emit_stats_fold(0)
            emit_stats_finish(0, pg0)

            # ---------------- software-pipelined frame loop ----------------
            for f in range(FPC):
                x_sb = x_tiles[f]
                a_sb, b_sb = ab_tiles[f]

                if f + 1 < FPC:
                    emit_x_load(f + 1)
                    emit_stats_dve(f + 1)

                # normalize: h = a*x + b (bf16)
                h_sb = hp.tile([128, NCH, HW], BF16)
                for ci in range(NCH):
                    nc.scalar.activation(
                        out=h_sb[:, ci, :], in_=x_sb[:, ci, :], func=Identity,
                        bias=b_sb[:, ci:ci + 1], scale=a_sb[:, ci:ci + 1])

                # scores[p, s] = sum_c h[c, p] kq[c, s]  (+ mask row via K=1 matmul)
                psum_s = psB.tile([128, 8, S], F32, tag="ps_small")
                for j in range(8):
                    for ci in range(NCH):
                        nc.tensor.matmul(
                            psum_s[:, j, :],
                            lhsT=h_sb[:, ci, j * 128:(j + 1) * 128],
                            rhs=kqT_sb[:, ci, :],
                            start=(ci == 0), stop=False,
                        )
                    nc.tensor.matmul(
                        psum_s[:, j, :], lhsT=ones_col[:],
                        rhs=mask_bf[:, f, :], start=False, stop=True,
                    )

                psum_g_next = emit_stats_fold(f + 1) if f + 1 < FPC else None

                # softmax over s (no max-subtraction: |scale*scores| is small)
                p_sb = small.tile([128, 8, S], F32)
                nc.scalar.activation(out=p_sb[:], in_=psum_s[:], func=Exp, scale=SCALE)
                l8 = small.tile([128, 8, 1], F32)
                nc.vector.reduce_sum(l8[:], p_sb[:], axis=mybir.AxisListType.X)
                linv = small.tile([128, 8, 1], F32)
                nc.vector.reciprocal(linv[:], l8[:])
                p_bf = small.tile([128, 8, S], BF16)
                nc.vector.tensor_mul(p_bf[:], p_sb[:], linv[:].to_broadcast((128, 8, S)))

                # transpose weights to [s, q] for the output contraction
                psum_wT = psB.tile([64, 8, 128], BF16, tag="ps_small")
                for j in range(8):
                    nc.tensor.transpose(psum_wT[:, j, :], p_bf[:, j, :], identity[:])
                wT_sb = small.tile([64, 8, 128], BF16)
                nc.scalar.activation(out=wT_sb[:], in_=psum_wT[:], func=Copy)
                wT_flat = wT_sb[:].rearrange("p a b -> p (a b)")  # [64, 1024]

                if psum_g_next is not None:
                    emit_stats_finish(f + 1, psum_g_next)

                # out[oc, p] = sum_s vo[s, oc] w[p, s]  (+ bo) ; residual in-place
                for oc in range(NCH):
                    psum_o = psO.tile([128, 2, 512], F32, tag="ps_o")
                    for half in range(2):
                        nc.tensor.matmul(
                            psum_o[:, half, :],
                            lhsT=vo_bf[:, oc * 128:(oc + 1) * 128],
                            rhs=wT_flat[:, half * 512:(half + 1) * 512],
                            start=True, stop=not with_bo,
                        )
                        if with_bo:
                            nc.tensor.matmul(
                                psum_o[:, half, :],
                                lhsT=bo_bf[:, oc * 128:(oc + 1) * 128],
                                rhs=ones512[:], start=False, stop=True,
                            )
                    nc.vector.tensor_add(
                        x_sb[:, oc, :],
                        psum_o[:].rearrange("p a b -> p (a b)"),
                        x_sb[:, oc, :])

                nc.sync.dma_start(out=_chunked(out_d[:, f, :]), in_=x_sb[:])

    nc.finalize()
    return nc


def _prep_in_maps(x, context, gamma, beta, wq, bq, wkv, bkv, wo, bo):
    f32 = lambda a: np.ascontiguousarray(np.asarray(a, dtype=np.float32))
    x, context = f32(x), f32(context)
    # NOT transposed: kq build contracts over wq's output index (see _build)
    wqT = f32(np.asarray(wq, np.float32))
    wkvT = f32(np.asarray(wkv, np.float32).T)
    woT = f32(np.asarray(wo, np.float32).T)
    gammaT = f32(np.asarray(gamma, np.float32).reshape(NCH, 128).T)
    betaT = f32(np.asarray(beta, np.float32).reshape(NCH, 128).T)
    bqT = f32(np.asarray(bq, np.float32).reshape(NCH, 128).T)
    bkv_r = f32(np.asarray(bkv, np.float32).reshape(1, 2 * C))
    bo_r = f32(np.asarray(bo, np.float32).reshape(1, C))

    gmat = np.zeros((128, 8), np.float32)
    gmat[np.arange(128), np.arange(128) // CPG] = 1.0 / CPG
    emat = np.zeros((8, 128), np.float32)
    emat[np.arange(128) // CPG, np.arange(128)] = 1.0

    in_maps = []
    for core in range(NCORES):
        b, r = divmod(core, 4)
        xs = np.ascontiguousarray(x[b, :, r::4, :, :].reshape(C, FPC, HW))
        ctxT = np.ascontiguousarray(context[b].T)
        mask = np.zeros((1, FPC, S), np.float32)
        for f in range(FPC):
            t = 4 * f + r
            lim = min(4 * (t + 1), S)
            mask[0, f, lim:] = NEGINF
        in_maps.append(dict(
            x=xs, ctxT=ctxT, wqT=wqT, wkvT=wkvT, woT=woT,
            gammaT=gammaT, betaT=betaT, bqT=bqT, bkv=bkv_r, bo=bo_r,
            mask=mask, gmat=gmat, emat=emat,
        ))
    return in_maps


def kernel(x, context, gamma, beta, wq, bq, wkv, bkv, wo, bo,
           _trace=False, **_trace_kwargs):
    global LAST_RESULT
    with_bq = bool(np.any(np.asarray(bq)))
    with_bkv = bool(np.any(np.asarray(bkv)))
    with_bo = bool(np.any(np.asarray(bo)))
    key = (with_bq, with_bkv, with_bo)
    if key not in _GRAPH_CACHE:
        _GRAPH_CACHE[key] = _build(*key)
    nc = _GRAPH_CACHE[key]

    in_maps = _prep_in_maps(x, context, gamma, beta, wq, bq, wkv, bkv, wo, bo)
    res = run_bass_kernel_spmd(nc, in_maps, core_ids=list(range(NCORES)),
                               trace=_trace, **_trace_kwargs)
    LAST_RESULT = res

    out = np.empty((B, C, T, H, W), np.float32)
    for core in range(NCORES):
        b, r = divmod(core, 4)
        out[b, :, r::4, :, :] = res.results[core]["out"].reshape(C, FPC, H, W)
    return out


# revision 11
# speedup vs baseline: 1.6561x; 1.0010x over previous
"""Trainium2 Bass kernel: CausalCrossAttention (GroupNorm + Q proj + block-causal
cross-attention over a small context + out proj + residual).

Sharding: 8 cores, each owns one (batch b, frame-residue r) pair:
  b = core // 4, r = core % 4, frames t = r + 4*f for f in 0..3.
GroupNorm normalizes each (b, t) frame independently over (16ch x H*W), and
attention key/value come from the (tiny) per-batch context, so every core's
work is fully local -- no collectives.  The block-causal mask is shipped as a
per-core additive bias and applied inside PSUM via a rank-1 (K=1) matmul, so
all cores run the identical SPMD graph.

Key algebraic fusion (exact, by associativity): with S=64 << H*W=1024 the
projections fold into the context side:
    scores = (Wq h)^T k  = h^T (Wq^T k)  = h^T kq,      kq = Wq^T k   [C, S]
    out    = Wo (v^T w)  = (Wo v^T) w    = vo^T w,      vo = v Wo^T   [S, C]
kq / vo / k / v are tiny per-core constants computed once from the context,
so the per-frame work is one [C x S] contraction + softmax + one [S x C]
contraction -- ~9x fewer FLOPs than materializing q and the o-projection.

All heavy matmuls run in bf16 (f32 PSUM accumulation); GroupNorm statistics in
f32 (bn_stats/bn_aggr + tiny f32 matmuls to fold/expand 16-channel groups
across partitions).  rsqrt(var+eps) is computed with the bit-trick + 2 Newton
steps entirely on the VectorEngine so the ScalarEngine only ever needs one
activation table set (Copy/Identity/Exp).  The frame loop is software
pipelined: frame f+1's x-load and statistics interleave with frame f.
"""

import numpy as np

import concourse.bass as bass
import concourse.bacc as bacc
import concourse.mybir as mybir
import concourse.tile as tile
from concourse.bass_utils import run_bass_kernel_spmd
from concourse.masks import make_identity

# Problem shape (fixed by the harness).
B, C, T, H, W = 2, 512, 16, 32, 32
HW = H * W            # 1024 query positions per frame
S, D = 64, 1024       # context length, context dim
G = 32                # groupnorm groups
CPG = C // G          # 16 channels per group
NCORES = 8
FPC = (B * T) // NCORES   # 4 frames per core
NCH = C // 128        # 4 channel chunks of 128
NDCH = D // 128       # 8 context-dim chunks
EPS = 1e-5
SCALE = float(C) ** -0.5
NEGINF = -1e9
# quake rsqrt seed magic, pre-adjusted for taking bits of 0.5*x instead of x
MAGIC_HALF = 0x5F3759DF - 0x00400000

F32 = mybir.dt.float32
BF16 = mybir.dt.bfloat16
I32 = mybir.dt.int32

Identity = mybir.ActivationFunctionType.Identity
Copy = mybir.ActivationFunctionType.Copy
Exp = mybir.ActivationFunctionType.Exp
Alu = mybir.AluOpType

LAST_RESULT = None        # BassKernelResults of the most recent run (for test.py)
_GRAPH_CACHE = {}


def _chunked(dram_ap):
    """[N*128, ...] dram AP -> [128, N, ...] with channel = n*128 + p."""
    return dram_ap.rearrange("(a p) w -> p a w", p=128)


def _build(with_bq: bool, with_bkv: bool, with_bo: bool) -> bass.Bass:
    nc = bacc.Bacc()

    x_d = nc.declare_dram_parameter("x", [C, FPC, HW], F32, isOutput=False)
    ctxT_d = nc.declare_dram_parameter("ctxT", [D, S], F32, isOutput=False)
    wqT_d = nc.declare_dram_parameter("wqT", [C, C], F32, isOutput=False)
    wkvT_d = nc.declare_dram_parameter("wkvT", [D, 2 * C], F32, isOutput=False)
    woT_d = nc.declare_dram_parameter("woT", [C, C], F32, isOutput=False)
    gammaT_d = nc.declare_dram_parameter("gammaT", [128, NCH], F32, isOutput=False)
    betaT_d = nc.declare_dram_parameter("betaT", [128, NCH], F32, isOutput=False)
    bqT_d = nc.declare_dram_parameter("bqT", [128, NCH], F32, isOutput=False)
    bkv_d = nc.declare_dram_parameter("bkv", [1, 2 * C], F32, isOutput=False)
    bo_d = nc.declare_dram_parameter("bo", [1, C], F32, isOutput=False)
    mask_d = nc.declare_dram_parameter("mask", [S, FPC], F32, isOutput=False)
    gmat_d = nc.declare_dram_parameter("gmat", [128, 8], F32, isOutput=False)
    emat_d = nc.declare_dram_parameter("emat", [8, 128], F32, isOutput=False)
    out_d = nc.declare_dram_parameter("out", [C, FPC, HW], F32, isOutput=True)

    with tile.TileContext(nc) as tc:
        with (
            tc.tile_pool(name="consts", bufs=1) as wp,
            tc.tile_pool(name="wtmp", bufs=1) as wtmp,
            tc.tile_pool(name="stage", bufs=4) as stage,
            tc.tile_pool(name="xp", bufs=3) as xp,
            tc.tile_pool(name="hp", bufs=2) as hp,
            tc.tile_pool(name="small", bufs=2) as small,
            tc.tile_pool(name="psO", bufs=2, space="PSUM") as psO,
            tc.tile_pool(name="psB", bufs=2, space="PSUM") as psB,
        ):
            # ---------------- constants ----------------
            gammaT_sb = wp.tile([128, NCH], F32)
            betaT_sb = wp.tile([128, NCH], F32)
            bqT_sb = wp.tile([128, NCH], F32)
            gmat_sb = wp.tile([128, 8], F32)
            emat_sb = wp.tile([8, 128], F32)
            maskc_sb = wp.tile([S, FPC], F32)
            identity = wp.tile([128, 128], BF16)
            id_f32 = wp.tile([64, 64], F32)
            magic_sb = wp.tile([8, NCH], I32)

            nc.sync.dma_start(out=gammaT_sb[:], in_=gammaT_d[:, :])
            nc.sync.dma_start(out=betaT_sb[:], in_=betaT_d[:, :])
            nc.sync.dma_start(out=bqT_sb[:], in_=bqT_d[:, :])
            nc.sync.dma_start(out=gmat_sb[:], in_=gmat_d[:, :])
            nc.sync.dma_start(out=emat_sb[:], in_=emat_d[:, :])
            nc.sync.dma_start(out=maskc_sb[:], in_=mask_d[:, :])
            make_identity(nc, identity[:])
            make_identity(nc, id_f32[:])
            nc.gpsimd.memset(magic_sb[:], MAGIC_HALF)

            # ---------------- frame 0 x-load + DVE statistics (early) -------------
            x_tiles = [None] * FPC
            ab_tiles = [None] * FPC
            mv_tiles = [None] * FPC

            def emit_x_load(f):
                x_sb = xp.tile([128, NCH, HW], F32)
                nc.sync.dma_start(out=x_sb[:], in_=_chunked(x_d[:, f, :]))
                x_tiles[f] = x_sb

            def emit_stats_dve(f):
                """Per-channel mean / E[x^2] via bn_stats (DVE only)."""
                x_sb = x_tiles[f]
                st6 = small.tile([128, NCH, 2, 6], F32)
                mv = small.tile([128, NCH, 2], F32)
                for ci in range(NCH):
                    xv = x_sb[:, ci, :].rearrange("p (a b) -> p a b", a=2)
                    for k2 in range(2):
                        nc.vector.bn_stats(out=st6[:, ci, k2, :], in_=xv[:, k2, :])
                    nc.vector.bn_aggr(out=mv[:, ci, :], in_=st6[:, ci, :, :])
                msq = small.tile([128, NCH], F32)
                nc.vector.tensor_mul(msq[:], mv[:, :, 0], mv[:, :, 0])
                nc.vector.tensor_add(mv[:, :, 1], mv[:, :, 1], msq[:])
                mv_tiles[f] = mv

            def emit_stats_fold(f):
                psum_g = psB.tile([8, 8], F32, tag="ps_small")
                nc.tensor.matmul(
                    psum_g[:], lhsT=gmat_sb[:],
                    rhs=mv_tiles[f][:].rearrange("p a b -> p (a b)"),
                    start=True, stop=True,
                )
                return psum_g

            def emit_stats_finish(f, psum_g):
                """rsqrt via bit trick + 2 Newton steps (DVE), expand to channels,
                produce per-channel affine (a, b)."""
                gs = small.tile([8, NCH, 2], F32)
                nc.vector.tensor_copy(
                    out=gs[:], in_=psum_g[:].rearrange("p (a b) -> p a b", a=NCH))
                gsq = small.tile([8, NCH], F32)
                nc.vector.tensor_mul(gsq[:], gs[:, :, 0], gs[:, :, 0])
                hx = small.tile([8, NCH], F32)
                nc.vector.tensor_sub(hx[:], gs[:, :, 1], gsq[:])
                nc.vector.tensor_scalar(
                    out=hx[:], in0=hx[:], scalar1=EPS, scalar2=0.5,
                    op0=Alu.add, op1=Alu.mult)
                ya = small.tile([8, NCH], F32)
                yb = small.tile([8, NCH], F32)
                sh = small.tile([8, NCH], I32)
                nc.vector.tensor_scalar(
                    out=sh[:], in0=hx[:].bitcast(I32), scalar1=1, scalar2=None,
                    op0=Alu.arith_shift_right)
                nc.vector.tensor_sub(ya[:].bitcast(I32), magic_sb[:], sh[:])
                u = small.tile([8, NCH], F32)
                cur, nxt = ya, yb
                for _ in range(2):
                    nc.vector.tensor_mul(u[:], cur[:], cur[:])
                    nc.vector.tensor_mul(u[:], u[:], hx[:])
                    nc.vector.scalar_tensor_tensor(
                        out=nxt[:], in0=u[:], scalar=1.5, in1=cur[:],
                        op0=Alu.subtract, op1=Alu.mult)
                    cur, nxt = nxt, cur
                nc.vector.tensor_copy(out=gs[:, :, 1], in_=cur[:])
                psum_e = psB.tile([128, NCH, 2], F32, tag="ps_small")
                nc.tensor.matmul(
                    psum_e[:].rearrange("p a b -> p (a b)"),
                    lhsT=emat_sb[:], rhs=gs[:].rearrange("p a b -> p (a b)"),
                    start=True, stop=True,
                )
                a_sb = small.tile([128, NCH], F32)
                t_sb = small.tile([128, NCH], F32)
                b_sb = small.tile([128, NCH], F32)
                nc.vector.tensor_mul(a_sb[:], psum_e[:, :, 1], gammaT_sb[:])
                nc.vector.tensor_mul(t_sb[:], psum_e[:, :, 0], a_sb[:])
                nc.vector.tensor_sub(b_sb[:], betaT_sb[:], t_sb[:])
                ab_tiles[f] = (a_sb, b_sb)

            emit_x_load(0)
            emit_stats_dve(0)

            # ---------------- weights: DMA f32 stage -> bf16 cast ----------------
            wq_bf = wtmp.tile([128, NCH, C], BF16)
            wkv_bf = wtmp.tile([128, NDCH, 2 * C], BF16)
            wo_bf = wtmp.tile([128, NCH, C], BF16)
            ctx_bf = wtmp.tile([128, NDCH, S], BF16)

            def load_cast(dst_slice, src_ap, eng):
                st = stage.tile([128, 1024], F32, tag="stage")
                stv = st[:].rearrange("p (a w) -> p a w", w=src_ap.shape[-1])
                n = src_ap.shape[1]
                nc.sync.dma_start(out=stv[:, :n, :], in_=src_ap)
                if eng == "v":
                    nc.vector.tensor_copy(out=dst_slice, in_=stv[:, :n, :])
                else:
                    nc.scalar.activation(out=dst_slice, in_=stv[:, :n, :], func=Copy)

            wqT_c = _chunked(wqT_d[:, :])      # [128, 4, 512]
            woT_c = _chunked(woT_d[:, :])      # [128, 4, 512]
            wkvT_c = _chunked(wkvT_d[:, :])    # [128, 8, 1024]
            ctxT_c = _chunked(ctxT_d[:, :])    # [128, 8, 64]

            load_cast(ctx_bf[:], ctxT_c, "s")
            for i in range(2):
                load_cast(wq_bf[:, 2 * i:2 * i + 2, :], wqT_c[:, 2 * i:2 * i + 2, :],
                          "v" if i == 0 else "s")
            for i in range(NDCH):
                load_cast(wkv_bf[:, i:i + 1, :], wkvT_c[:, i:i + 1, :],
                          "v" if i % 2 == 0 else "s")
            for i in range(2):
                load_cast(wo_bf[:, 2 * i:2 * i + 2, :], woT_c[:, 2 * i:2 * i + 2, :],
                          "v" if i == 0 else "s")

            if with_bkv:
                ones64 = wp.tile([1, S], BF16)
                nc.vector.memset(ones64[:], 1.0)
                stb = small.tile([1, 2 * C], F32)
                nc.sync.dma_start(out=stb[:], in_=bkv_d[:, :])
                bkv_bf = wp.tile([1, 2 * C], BF16)
                nc.vector.tensor_copy(out=bkv_bf[:], in_=stb[:])
            if with_bo:
                ones512 = wp.tile([1, 512], BF16)
                nc.vector.memset(ones512[:], 1.0)
                sbo = small.tile([1, C], F32)
                nc.sync.dma_start(out=sbo[:], in_=bo_d[:, :])
                bo_bf = wp.tile([1, C], BF16)
                nc.vector.tensor_copy(out=bo_bf[:], in_=sbo[:])

            # ---------------- context-side constants: k, v, kq, vo ---------------
            # kv[s, o] = sum_d ctx[s, d] wkv[o, d] (+ bkv); k = kv[:, :C], v = kv[:, C:]
            kT_sb = small.tile([128, NCH, S], BF16)   # k^T: [c, s]
            vT_sb = small.tile([128, NCH, S], BF16)   # v^T: [c, s]
            for half in range(2):
                psum_kv = psB.tile([64, 512], F32, tag="ps_small")
                for dci in range(NDCH):
                    nc.tensor.matmul(
                        psum_kv[:],
                        lhsT=ctx_bf[:, dci, :],
                        rhs=wkv_bf[:, dci, half * 512:(half + 1) * 512],
                        start=(dci == 0),
                        stop=(dci == NDCH - 1 and not with_bkv),
                    )
                if with_bkv:
                    nc.tensor.matmul(
                        psum_kv[:], lhsT=ones64[:],
                        rhs=bkv_bf[:, half * 512:(half + 1) * 512],
                        start=False, stop=True,
                    )
                kv_sb = small.tile([64, 512], BF16)
                nc.scalar.activation(out=kv_sb[:], in_=psum_kv[:], func=Copy)
                psum_t = psB.tile([128, NCH, S], BF16, tag="ps_small")
                for ci in range(NCH):
                    nc.tensor.transpose(
                        psum_t[:, ci, :],
                        kv_sb[:, ci * 128:(ci + 1) * 128],
                        identity[:64, :64],
                    )
                dst = kT_sb if half == 0 else vT_sb
                nc.scalar.activation(out=dst[:], in_=psum_t[:], func=Copy)

            # scores need m[c, s] = sum_c' wq[c', c] k[s, c']  (contraction over
            # Wq's OUTPUT index), so the stationary operand is wq in NATURAL
            # layout: lhsT[K=c'-chunk, M=c-tile] = wq[c', c].  The "wqT" dram
            # param therefore carries wq UNtransposed (see _prep_in_maps).
            kqT_sb = wp.tile([128, NCH, S], BF16)
            psum_kq = psB.tile([128, NCH, S], F32, tag="ps_small")
            for co in range(NCH):
                for ci in range(NCH):
                    nc.tensor.matmul(
                        psum_kq[:, co, :],
                        lhsT=wq_bf[:, ci, co * 128:(co + 1) * 128],
                        rhs=kT_sb[:, ci, :],
                        start=(ci == 0), stop=(ci == NCH - 1),
                    )
            nc.scalar.activation(out=kqT_sb[:], in_=psum_kq[:], func=Copy)

            # vo[s, oc] = sum_c v[s, c] wo[oc, c] = sum_c vT[c, s]^T woT[c, oc]
            vo_bf = wp.tile([64, C], BF16)
            psum_vo = psB.tile([64, 512], F32, tag="ps_small")
            for ci in range(NCH):
                nc.tensor.matmul(
                    psum_vo[:],
                    lhsT=vT_sb[:, ci, :],
                    rhs=wo_bf[:, ci, :],
                    start=(ci == 0), stop=(ci == NCH - 1),
                )
            nc.scalar.activation(out=vo_bf[:], in_=psum_vo[:], func=Copy)

            # bqk[s] = sum_c' bq[c'] k[s, c']: fold q-bias into the mask column
            if with_bq:
                psum_bq = psB.tile([S, 1], F32, tag="ps_small")
                for ci in range(NCH):
                    nc.tensor.matmul(
                        psum_bq[:],
                        lhsT=kT_sb[:, ci, :],
                        rhs=bqT_sb[:, ci:ci + 1],
                        start=(ci == 0), stop=(ci == NCH - 1),
                    )
                nc.vector.tensor_add(
                    maskc_sb[:], maskc_sb[:],
                    psum_bq[:].to_broadcast((S, FPC)))

            # frame 0 statistics fold/finish (tiny PE + DVE)
            pg0 = emit_stats_fold(0)
            emit_stats_finish(0, pg0)

            # ---------------- software-pipelined frame loop ----------------
            for f in range(FPC):
                x_sb = x_tiles[f]
                a_sb, b_sb = ab_tiles[f]

                if f + 1 < FPC:
                    emit_x_load(f + 1)
                    emit_stats_dve(f + 1)

                # normalize: h = a*x + b (bf16)
                h_sb = hp.tile([128, NCH, HW], BF16)
                for ci in range(NCH):
                    nc.scalar.activation(
                        out=h_sb[:, ci, :], in_=x_sb[:, ci, :], func=Identity,
                        bias=b_sb[:, ci:ci + 1], scale=a_sb[:, ci:ci + 1])

                # scoresT[s, p] = sum_c kq[c, s] h[c, p]  (dense N=512 matmuls)
                psum_scT = psO.tile([S, 2, 512], F32, tag="ps_sct", bufs=1)
                for half in range(2):
                    for ci in range(NCH):
                        nc.tensor.matmul(
                            psum_scT[:, half, :],
                            lhsT=kqT_sb[:, ci, :],
                            rhs=h_sb[:, ci, half * 512:(half + 1) * 512],
                            start=(ci == 0), stop=(ci == NCH - 1),
                        )
                # PSUM -> SBUF with the causal mask applied as per-partition bias
                scT_sb = small.tile([S, 2, 512], F32)
                nc.scalar.activation(
                    out=scT_sb[:], in_=psum_scT[:], func=Identity,
                    bias=maskc_sb[:, f:f + 1], scale=1.0)
                scT_flat = scT_sb[:].rearrange("p a b -> p (a b)")  # [64, 1024]
                # transpose back to [p, s] tiles for free-axis softmax
                psum_s = psB.tile([128, 8, S], F32, tag="ps_small")
                for j in range(8):
                    nc.tensor.transpose(
                        psum_s[:, j, :], scT_flat[:, j * 128:(j + 1) * 128],
                        id_f32[:])

                psum_g_next = emit_stats_fold(f + 1) if f + 1 < FPC else None

                # softmax over s (no max-subtraction: |scale*scores| is small)
                p_sb = small.tile([128, 8, S], F32)
                nc.scalar.activation(out=p_sb[:], in_=psum_s[:], func=Exp, scale=SCALE)
                l8 = small.tile([128, 8, 1], F32)
                nc.vector.reduce_sum(l8[:], p_sb[:], axis=mybir.AxisListType.X)
                linv = small.tile([128, 8, 1], F32)
                nc.vector.reciprocal(linv[:], l8[:])
                p_bf = small.tile([128, 8, S], BF16)
                nc.vector.tensor_mul(p_bf[:], p_sb[:], linv[:].to_broadcast((128, 8, S)))

                # transpose weights to [s, q] for the output contraction
                psum_wT = psB.tile([64, 8, 128], BF16, tag="ps_small")
                for j in range(8):
                    nc.tensor.transpose(psum_wT[:, j, :], p_bf[:, j, :], identity[:])
                wT_sb = small.tile([64, 8, 128], BF16)
                nc.scalar.activation(out=wT_sb[:], in_=psum_wT[:], func=Copy)
                wT_flat = wT_sb[:].rearrange("p a b -> p (a b)")  # [64, 1024]

                if psum_g_next is not None:
                    emit_stats_finish(f + 1, psum_g_next)

                # out[oc, p] = sum_s vo[s, oc] w[p, s]  (+ bo) ; residual in-place
                for oc in range(NCH):
                    psum_o = psO.tile([128, 2, 512], F32, tag="ps_o")
                    for half in range(2):
                        nc.tensor.matmul(
                            psum_o[:, half, :],
                            lhsT=vo_bf[:, oc * 128:(oc + 1) * 128],
                            rhs=wT_flat[:, half * 512:(half + 1) * 512],
                            start=True, stop=not with_bo,
                        )
                        if with_bo:
                            nc.tensor.matmul(
                                psum_o[:, half, :],
                                lhsT=bo_bf[:, oc * 128:(oc + 1) * 128],
                                rhs=ones512[:], start=False, stop=True,
                            )
                    nc.vector.tensor_add(
                        x_sb[:, oc, :],
                        psum_o[:].rearrange("p a b -> p (a b)"),
                        x_sb[:, oc, :])

                nc.sync.dma_start(out=_chunked(out_d[:, f, :]), in_=x_sb[:])

    nc.finalize()
    return nc


def _prep_in_maps(x, context, gamma, beta, wq, bq, wkv, bkv, wo, bo):
    f32 = lambda a: np.ascontiguousarray(np.asarray(a, dtype=np.float32))
    x, context = f32(x), f32(context)
    # NOT transposed: kq build contracts over wq's output index (see _build)
    wqT = f32(np.asarray(wq, np.float32))
    wkvT = f32(np.asarray(wkv, np.float32).T)
    woT = f32(np.asarray(wo, np.float32).T)
    gammaT = f32(np.asarray(gamma, np.float32).reshape(NCH, 128).T)
    betaT = f32(np.asarray(beta, np.float32).reshape(NCH, 128).T)
    bqT = f32(np.asarray(bq, np.float32).reshape(NCH, 128).T)
    bkv_r = f32(np.asarray(bkv, np.float32).reshape(1, 2 * C))
    bo_r = f32(np.asarray(bo, np.float32).reshape(1, C))

    gmat = np.zeros((128, 8), np.float32)
    gmat[np.arange(128), np.arange(128) // CPG] = 1.0 / CPG
    emat = np.zeros((8, 128), np.float32)
    emat[np.arange(128) // CPG, np.arange(128)] = 1.0

    in_maps = []
    for core in range(NCORES):
        b, r = divmod(core, 4)
        xs = np.ascontiguousarray(x[b, :, r::4, :, :].reshape(C, FPC, HW))
        ctxT = np.ascontiguousarray(context[b].T)
        mask = np.zeros((S, FPC), np.float32)
        for f in range(FPC):
            t = 4 * f + r
            lim = min(4 * (t + 1), S)
            mask[lim:, f] = NEGINF
        in_maps.append(dict(
            x=xs, ctxT=ctxT, wqT=wqT, wkvT=wkvT, woT=woT,
            gammaT=gammaT, betaT=betaT, bqT=bqT, bkv=bkv_r, bo=bo_r,
            mask=mask, gmat=gmat, emat=emat,
        ))
    return in_maps


def kernel(x, context, gamma, beta, wq, bq, wkv, bkv, wo, bo,
           _trace=False, **_trace_kwargs):
    global LAST_RESULT
    with_bq = bool(np.any(np.asarray(bq)))
    with_bkv = bool(np.any(np.asarray(bkv)))
    with_bo = bool(np.any(np.asarray(bo)))
    key = (with_bq, with_bkv, with_bo)
    if key not in _GRAPH_CACHE:
        _GRAPH_CACHE[key] = _build(*key)
    nc = _GRAPH_CACHE[key]

    in_maps = _prep_in_maps(x, context, gamma, beta, wq, bq, wkv, bkv, wo, bo)
    res = run_bass_kernel_spmd(nc, in_maps, core_ids=list(range(NCORES)),
                               trace=_trace, **_trace_kwargs)
    LAST_RESULT = res

    out = np.empty((B, C, T, H, W), np.float32)
    for core in range(NCORES):
        b, r = divmod(core, 4)
        out[b, :, r::4, :, :] = res.results[core]["out"].reshape(C, FPC, H, W)
    return out
